# revision 43
# baseline (speedup 1.0000x reference)
"""TRN2 Bass kernel for nn_Block_58574763983799 (dense transformer block), v4.

Self-contained: builds/compiles the Bass program on first call (cached),
shards the batch (data-parallel) over 8 NeuronCores, returns fp32.
Measured (CoreSim cost model, one core, 16384 tokens): ~776 us;
HW rel err 1.74e-2 (gate 2e-2).

Key structure (vs the v2 baseline at ~1060 us):
  - FFN1 in fp8 e4m3 with DoubleRow perf mode (K=256 per pass, 0.5
    cyc/row): W1 host-scaled x32 (descale folded into the relu scale);
    the odd K-tile rides a stride-0 broadcast rhs pair.  FFN2 stays f16
    (fp8 there pushes rel err past the gate).
  - x / Wp / W2 host-scaled x16 so the output residual add needs no
    rescale op; the host divides the (f16, x16) output by 16.  LN is
    scale-invariant so the residual scale is free.
  - softmax: per-4-head score bank, mask-add on DVE writing f16 ngm
    (masked -> +inf/sat, releases the PSUM bank), per-head windowed
    min-reduce (DVE), Pool broadcast-subtract of the per-head max in
    SBUF, one Act exp per bank -> bf16 weights.  Per-head max keeps
    every head's bf16 weights in range (sums >= 1, no NaN rcp).
  - softmax denominators via ones-column matmuls off the transposed
    weights; normalization folded into the attn PSUM->SBUF drain as a
    single DVE tensor_tensor with a stride-0 broadcast reciprocal.
  - output written channel-major [C, n_tok] (no final transposes).
  - engine placement honors walrus rules (GPSIMD cannot touch PSUM,
    TensorTensorReduce/int-shift unsupported): PSUM drains live on
    Act/DVE, Pool does SBUF-side work (LN apply/newton, h2T8 convert,
    per-head max subtract), transposes + DMA on SP.
  - PE p-state ramp: score-bank matmuls are emitted interleaved with
    the qk projection so the PE queue never drains; softmax and LN run
    1-2 stages behind/ahead in a 6-deep software pipeline.

  kernel(**inputs) -> np.ndarray [2048, 64, 384] float32
"""
import sys
if "/opt/trn_rl_repo" not in sys.path:
    sys.path.insert(0, "/opt/trn_rl_repo")
import numpy as np
import ml_dtypes

import concourse.bass as bass
import concourse.mybir as mybir
import concourse.tile as tile
from contextlib import ExitStack
from concourse.vector_clock import ScopedClock, VectorClock

F32 = mybir.dt.float32
F16 = mybir.dt.float16
BF16 = mybir.dt.bfloat16
FP8 = mybir.dt.float8e4
AF = mybir.ActivationFunctionType
ALU = mybir.AluOpType
DR = mybir.MatmulPerfMode.DoubleRow

C = 384
H = 8
D = 48
DP = 64          # padded head dim (q/k only)
T = 64           # tokens per attention block
FF = 4 * C       # 1536
CP = H * DP      # 512 padded q/k concat dim
RP = 128         # rowpair tokens
ST = 512         # supertile tokens
NEG = -1.0e9
SCALE = D * (C ** -0.5)
EPS = 1e-5
MAGIC = 0x5F3759DF
XS = 16.0        # x / Wp host scale (folded out in output ttr)
S1 = 32.0        # W1 fp8 scale (folded out in relu scale)
S2 = 16.0        # W2 fp8 scale (folded out in output ttr; must equal XS)


def patch_drain():
    """Split the TileContext tail-drain's semaphore waits into 1-wait NOPs."""
    if getattr(tile.TileContext, "_drain_patched", False):
        return

    def _drain_and_barrier(self, tick_clock, wait_clock):
        nc = self.nc
        gcv = tick_clock.global_clock
        ticks = list(gcv)
        nz = [(i, t) for i, t in enumerate(ticks) if t > 0]
        for i, t in nz:
            vec = [0] * len(ticks)
            vec[i] = t
            nop = nc.sync.nop()
            wait_clock.add_sem_waits(nop.ins, ScopedClock({None: VectorClock(vec)}))
        nc.sync.drain()
        nc.all_engine_barrier()
        assert self.sems is not None
        popped = nc._tile_sem_poison_stack.pop()
        assert popped is self._sem_poison
        nc.clear_and_free_semaphores(list(self.sems.allocated().values()))
        nc.all_engine_barrier()

    tile.TileContext._drain_and_barrier = _drain_and_barrier
    tile.TileContext._drain_patched = True


def to_fp8(a):
    return np.asarray(a, np.float32).astype(ml_dtypes.float8_e4m3fn)


def build_weights(inputs):
    """Host-side weight prep from the raw reference inputs (numpy f32)."""
    Wq = np.asarray(inputs["Wq"], np.float32)  # [H, C, D]
    Wk = np.asarray(inputs["Wk"], np.float32)
    Wv = np.asarray(inputs["Wv"], np.float32)
    Wp = np.asarray(inputs["Wp"], np.float32)  # [C, C]
    W1 = np.asarray(inputs["W1"], np.float32)  # [C, FF]
    W2 = np.asarray(inputs["W2"], np.float32)  # [FF, C]

    # wqk [C, 2*CP]: q heads 64-padded (SCALE folded into q); k same
    wqk = np.zeros((C, 2 * CP), np.float32)
    for h in range(H):
        wqk[:, DP * h : DP * h + D] = Wq[h] * SCALE
        wqk[:, CP + DP * h : CP + DP * h + D] = Wk[h]
    # wv [C, C] dense concat of per-head Wv
    wv = np.zeros((C, C), np.float32)
    for h in range(H):
        wv[:, D * h : D * h + D] = Wv[h]

    # negated block-causal additive mask: +1e9 masked, 0 valid, tiled 4x;
    # (maddn4 - scores) saturates to +inf in the f16 ngm tile -> exp -> 0
    maddn = np.full((RP, RP), -NEG, np.float32)
    for b in range(2):
        for tl in range(T):
            maddn[b * T + tl, b * T : b * T + tl + 1] = 0.0
    maddn4 = np.tile(maddn, (1, 4))

    # W1 fp8 x S1, DoubleRow pair layouts:
    #   w1a [128, 2, FF] = k-tiles (0, 1);  w1b [128, 2, FF] = (k-tile 2, 0)
    W1s = W1 * S1
    w1a = np.stack([W1s[0:128], W1s[128:256]], axis=1)           # [128, 2, FF]
    w1b = np.stack([W1s[256:384], np.zeros((128, FF), np.float32)], axis=1)
    # W2 f16 x S2, [128, 12, C]
    w2 = (W2 * S2).reshape(12, 128, C).transpose(1, 0, 2).copy()

    return {
        "wqk": wqk.astype(np.float16),
        "wv": wv.astype(np.float16),
        "wp": (Wp * XS).astype(np.float16),
        "w1a": to_fp8(w1a),
        "w1b": to_fp8(w1b),
        "w2": w2.astype(np.float16),
        "maddn4": maddn4,
        # bp joins the XS-scaled residual stream (xpb = XS*(x+bp))
        "bp": np.asarray(inputs["bp"], np.float32) * XS,
        "b1": np.asarray(inputs["b1"], np.float32),
        # b2 joins the XS-scaled output stream
        "b2": np.asarray(inputs["b2"], np.float32) * XS,
        "ln1g": np.asarray(inputs["ln1_g"], np.float32),
        "ln1b": np.asarray(inputs["ln1_b"], np.float32),
        "ln2g": np.asarray(inputs["ln2_g"], np.float32),
        "ln2b": np.asarray(inputs["ln2_b"], np.float32),
    }


def build_nc(n_tok, trivial_ln=True, trivial_bias=True, n_rep=1):
    """Build the Bass program for one core processing [n_tok, C] tokens.

    Input x is host-scaled by XS; output is channel-major [C, n_tok] f16.
    """
    patch_drain()
    assert n_tok % ST == 0
    n_st = n_tok // ST
    nc = bass.Bass()

    x_d = nc.dram_tensor("x", [n_tok, C], F16, kind="ExternalInput")
    wqk_d = nc.dram_tensor("wqk", [C, 2 * CP], F16, kind="ExternalInput")
    wv_d = nc.dram_tensor("wv", [C, C], F16, kind="ExternalInput")
    wp_d = nc.dram_tensor("wp", [C, C], F16, kind="ExternalInput")
    w1a_d = nc.dram_tensor("w1a", [128, 2, FF], FP8, kind="ExternalInput")
    w1b_d = nc.dram_tensor("w1b", [128, 2, FF], FP8, kind="ExternalInput")
    w2_d = nc.dram_tensor("w2", [128, 12, C], F16, kind="ExternalInput")
    maddn4_d = nc.dram_tensor("maddn4", [RP, 4 * RP], F32, kind="ExternalInput")
    bp_d = nc.dram_tensor("bp", [C], F32, kind="ExternalInput")
    b1_d = nc.dram_tensor("b1", [FF], F32, kind="ExternalInput")
    b2_d = nc.dram_tensor("b2", [C], F32, kind="ExternalInput")
    if not trivial_ln:
        ln1g_d = nc.dram_tensor("ln1g", [C], F32, kind="ExternalInput")
        ln1b_d = nc.dram_tensor("ln1b", [C], F32, kind="ExternalInput")
        ln2g_d = nc.dram_tensor("ln2g", [C], F32, kind="ExternalInput")
        ln2b_d = nc.dram_tensor("ln2b", [C], F32, kind="ExternalInput")
    out_d = nc.dram_tensor("out", [C, n_tok], F16, kind="ExternalOutput")

    with tile.TileContext(nc) as tc, ExitStack() as ctx:
        cpool = ctx.enter_context(tc.tile_pool(name="consts", bufs=1))

        # ---- constants into SBUF ----
        wqk_sb = cpool.tile([128, 3, 2 * CP], F16)
        nc.gpsimd.dma_start(out=wqk_sb, in_=wqk_d.rearrange("(a p) n -> p a n", p=128))
        wv_sb = cpool.tile([128, 3, C], F16)
        nc.scalar.dma_start(out=wv_sb, in_=wv_d.rearrange("(a p) n -> p a n", p=128))
        wp_sb = cpool.tile([128, 3, C], F16)
        nc.scalar.dma_start(out=wp_sb, in_=wp_d.rearrange("(a p) n -> p a n", p=128))
        w1a_sb = cpool.tile([128, 2, FF], FP8)
        nc.gpsimd.dma_start(out=w1a_sb, in_=w1a_d[:, :, :])
        w1b_sb = cpool.tile([128, 2, FF], FP8)
        nc.gpsimd.dma_start(out=w1b_sb, in_=w1b_d[:, :, :])
        w2_sb = cpool.tile([128, 12, C], F16)
        nc.gpsimd.dma_start(out=w2_sb, in_=w2_d[:, :, :])
        maddn4_sb = cpool.tile([128, 4 * RP], F32)
        nc.scalar.dma_start(out=maddn4_sb, in_=maddn4_d[:, :])
        ones_sb = cpool.tile([128, 1], BF16)
        nc.vector.memset(ones_sb, 1.0)
        magic_sb = cpool.tile([128, 4], mybir.dt.uint32)
        nc.vector.memset(magic_sb, MAGIC)
        b1_sb = cpool.tile([128, 12], F32)
        nc.gpsimd.dma_start(out=b1_sb, in_=b1_d.rearrange("(a p) -> p a", p=128))
        b2c_sb = cpool.tile([128, 3], F32)
        nc.gpsimd.dma_start(out=b2c_sb, in_=b2_d.rearrange("(a p) -> p a", p=128))
        bpb_sb = cpool.tile([128, C], F16)
        nc.gpsimd.dma_start(out=bpb_sb, in_=bp_d[None, :].to_broadcast([128, C]))
        if not trivial_ln:
            lnb = {}
            for nm, dten in (("ln1g", ln1g_d), ("ln1b", ln1b_d),
                             ("ln2g", ln2g_d), ("ln2b", ln2b_d)):
                t_ = cpool.tile([128, C], F16, tag=nm)
                nc.gpsimd.dma_start(out=t_, in_=dten[None, :].to_broadcast([128, C]))
                lnb[nm] = t_

        # ---- pools ----
        xin = ctx.enter_context(tc.tile_pool(name="xin", bufs=20))
        stat = ctx.enter_context(tc.tile_pool(name="stat", bufs=4))
        hbuf = ctx.enter_context(tc.tile_pool(name="hbuf", bufs=12))
        htp = ctx.enter_context(tc.tile_pool(name="htp", bufs=3))
        qkp = ctx.enter_context(tc.tile_pool(name="qkp", bufs=3, space="PSUM"))
        ffp = ctx.enter_context(tc.tile_pool(name="ffp", bufs=2, space="PSUM"))
        qks = ctx.enter_context(tc.tile_pool(name="qks", bufs=2))
        vsb = ctx.enter_context(tc.tile_pool(name="vsb", bufs=12))
        scp = ctx.enter_context(tc.tile_pool(name="scp", bufs=2, space="PSUM"))
        smx = ctx.enter_context(tc.tile_pool(name="smx", bufs=3))
        smv = ctx.enter_context(tc.tile_pool(name="smv", bufs=8))
        wtp = ctx.enter_context(tc.tile_pool(name="wtp", bufs=2))
        atp = ctx.enter_context(tc.tile_pool(name="atp", bufs=1, space="PSUM"))
        ats = ctx.enter_context(tc.tile_pool(name="ats", bufs=2))
        atT = ctx.enter_context(tc.tile_pool(name="atT", bufs=2))
        x2p = ctx.enter_context(tc.tile_pool(name="x2p", bufs=8))
        h2p = ctx.enter_context(tc.tile_pool(name="h2p", bufs=2))
        rlu = ctx.enter_context(tc.tile_pool(name="rlu", bufs=2))
        oub = ctx.enter_context(tc.tile_pool(name="oub", bufs=4))

        x_v = x_d.rearrange("(s p) c -> s p c", p=RP)      # [n_rp, 128, C]

        def layer_norm(src_tiles, gname, bname):
            """src_tiles: 4 SBUF [128, C] f16 tiles -> 4 normalized f16 tiles."""
            mv = stat.tile([128, 4, 2], F32, tag="mv", name="mv")
            for rp in range(4):
                bstat = stat.tile([128, 6], F32, tag="bstat", name="bstat")
                nc.vector.bn_stats(out=bstat, in_=src_tiles[rp])
                nc.vector.bn_aggr(out=mv[:, rp, :], in_=bstat)
            # rstd = rsqrt(var + eps), batched Newton on [128, 4]
            ve = stat.tile([128, 4], F32, tag="ve", name="ve")
            nc.gpsimd.tensor_scalar_add(ve, mv[:, :, 1], EPS)
            vh = stat.tile([128, 4], F32, tag="vh", name="vh")
            nc.gpsimd.tensor_scalar_mul(vh, ve, -0.5)
            ub = stat.tile([128, 4], mybir.dt.uint32, tag="ub", name="ub")
            nc.vector.tensor_scalar(
                out=ub, in0=ve.bitcast(mybir.dt.uint32), scalar1=1,
                scalar2=None, op0=ALU.logical_shift_right)
            y = stat.tile([128, 4], F32, tag="y", name="y")
            nc.vector.tensor_tensor(
                out=y.bitcast(mybir.dt.uint32), in0=magic_sb, in1=ub,
                op=ALU.subtract)
            tq = stat.tile([128, 4], F32, tag="tq", name="tq")
            for _ in range(2):
                nc.gpsimd.tensor_tensor(out=tq, in0=y, in1=y, op=ALU.mult)
                nc.gpsimd.tensor_tensor(out=tq, in0=tq, in1=vh, op=ALU.mult)
                nc.gpsimd.tensor_scalar_add(tq, tq, 1.5)
                nc.gpsimd.tensor_tensor(out=y, in0=y, in1=tq, op=ALU.mult)
            hs = []
            for rp in range(4):
                h_ = hbuf.tile([128, C], F16, tag="h", name="h")
                nc.gpsimd.tensor_scalar(
                    out=h_, in0=src_tiles[rp],
                    scalar1=mv[:, rp, 0:1], scalar2=y[:, rp : rp + 1],
                    op0=ALU.subtract, op1=ALU.mult)
                if not trivial_ln:
                    nc.gpsimd.tensor_tensor(out=h_, in0=h_, in1=lnb[gname], op=ALU.mult)
                    nc.gpsimd.tensor_tensor(out=h_, in0=h_, in1=lnb[bname], op=ALU.add)
                hs.append(h_)
            return hs

        def transpose_sb(h_tiles, dst):
            """4x [128, C] f16 -> dst [128, 3, ST] f16 via DMA-transpose."""
            for rp in range(4):
                for ct in range(3):
                    nc.sync.dma_start(
                        out=dst[:, ct, rp * RP : (rp + 1) * RP],
                        in_=h_tiles[rp][:, ct * 128 : (ct + 1) * 128],
                        transpose=True)

        xq = {}

        def load_x(st):
            x_t = []
            for rp in range(4):
                xt = xin.tile([RP, C], F16, tag="x", name="x")
                nc.sync.dma_start(out=xt, in_=x_v[st * 4 + rp])
                x_t.append(xt)
            xq[st] = x_t

        def front_ln(st):
            if st not in xq:
                load_x(st)
            if st + 1 < n_st:
                load_x(st + 1)
            x_t = xq.pop(st)

            h1 = layer_norm(x_t, "ln1g", "ln1b")
            h1T = htp.tile([128, 3, ST], F16, tag="h1T", name="h1T")
            transpose_sb(h1, h1T)

            if trivial_bias:
                xpb = x_t
            else:
                xpb = []
                for rp in range(4):
                    xp_ = xin.tile([RP, C], F16, tag="xpb", name="xpb")
                    nc.vector.tensor_tensor(out=xp_, in0=x_t[rp], in1=bpb_sb, op=ALU.add)
                    xpb.append(xp_)
            return dict(x_t=x_t, xpb=xpb, h1T=h1T)

        def emit_qk_tile(S, mt, fine=False):
            """qk projection tile mt (of 8): 3 matmuls + copy.

            fine=True splits the rhs per rowpair so the first tile can
            start as soon as rp0's h1T transposes land (warmup only)."""
            h1T = S["h1T"]
            ps = qkp.tile([128, ST], F32, tag="qkps", name="qkps")
            if fine:
                for rp in range(4):
                    for kt in range(3):
                        nc.tensor.matmul(
                            ps[:, rp * RP : (rp + 1) * RP],
                            wqk_sb[:, kt, mt * 128 : (mt + 1) * 128],
                            h1T[:, kt, rp * RP : (rp + 1) * RP],
                            start=(kt == 0), stop=(kt == 2),
                            skip_group_check=True)
            else:
                for kt in range(3):
                    nc.tensor.matmul(
                        ps, wqk_sb[:, kt, mt * 128 : (mt + 1) * 128],
                        h1T[:, kt, :], start=(kt == 0), stop=(kt == 2))
            sb = qks.tile([128, ST], F16, tag=f"qk{mt}", name=f"qk{mt}")
            nc.scalar.activation(out=sb, in_=ps, func=AF.Copy)
            S["qk_sb"].append(sb)

        def emit_score_bank(S, rp, half):
            """scores for 4 heads (one parity) of rowpair rp + fused softmax."""
            qk_sb = S["qk_sb"]
            tsl = slice(rp * RP, (rp + 1) * RP)
            b0 = DP * half
            sps = scp.tile([128, 4 * RP], F32, tag="scores", name="scores")
            for hh in range(4):
                nc.tensor.matmul(
                    sps[:, hh * RP : (hh + 1) * RP],
                    qk_sb[hh][b0 : b0 + DP, tsl],
                    qk_sb[4 + hh][b0 : b0 + DP, tsl],
                    start=(hh == 0), stop=(hh == 3),
                    tile_position=(b0, 0))
            # negated-mask add on DVE writing f16 (masked -> +inf), which
            # releases the bank; the min-reduce then runs in DVE 2x mode.
            ngm = smx.tile([128, 4 * RP], F16, tag="ngm", name="ngm", bufs=18)
            nc.vector.tensor_tensor(out=ngm, in0=maddn4_sb, in1=sps,
                                    op=ALU.subtract)
            S["banks"].append((rp, half, ngm))

        def emit_softmax(S):
            """per-head min-reduce (DVE) + Pool broadcast-subtract + exp.

            Per-head stabilization keeps every head's bf16 weights in
            range (sums >= 1 by construction -> no NaN reciprocals)."""
            for rp, half, ngm in S.pop("banks"):
                mnh = smv.tile([128, 4], F32, tag="mnh", name="mnh")
                nc.vector.tensor_reduce(
                    out=mnh, in_=ngm.rearrange("p (h k) -> p h k", h=4),
                    axis=mybir.AxisListType.X, op=ALU.min)
                ngm2 = smx.tile([128, 4 * RP], F16, tag="ngm2", name="ngm2")
                nc.gpsimd.tensor_tensor(
                    out=ngm2.rearrange("p (h k) -> p h k", h=4),
                    in0=ngm.rearrange("p (h k) -> p h k", h=4),
                    in1=mnh[:, :, None].to_broadcast([128, 4, RP]),
                    op=ALU.subtract)
                expw = smx.tile([128, 4 * RP], BF16, tag="expw", name="expw")
                nc.scalar.activation(out=expw, in_=ngm2, func=AF.Exp,
                                     bias=0.0, scale=-1.0)
                for hh in range(4):
                    h = 2 * hh + half
                    wT = wtp.tile([128, RP], BF16, tag=f"wT{rp}_{h}",
                                  name=f"wT{rp}_{h}")
                    nc.sync.dma_start(out=wT,
                                      in_=expw[:, hh * RP : (hh + 1) * RP],
                                      transpose=True)
                    S["weiT"][(rp, h)] = wT

        def front_v(st, S):
            """v projection (dense concat heads), bf16 out."""
            h1T = S["h1T"]
            v_sb = []
            for rp in range(4):
                ps = qkp.tile([RP, ST], F32, tag="qkps", name="qkps")
                for kt in range(3):
                    nc.tensor.matmul(
                        ps[:, :C], h1T[:, kt, rp * RP : (rp + 1) * RP],
                        wv_sb[:, kt, :], start=(kt == 0), stop=(kt == 2))
                sb = vsb.tile([RP, C], BF16, tag="v", name="v")
                nc.scalar.activation(out=sb, in_=ps[:, :C], func=AF.Copy)
                v_sb.append(sb)
            S["v_sb"] = v_sb

        def back1b(st, S):
            """attn@v + sums, normalize-copy, attnT, Wp, residual, LN2."""
            v_sb, xpb, weiT = S["v_sb"], S["xpb"], S["weiT"]

            attn_sb = []
            for rp in range(4):
                aps = atp.tile([128, H, D + 1], F32, tag="attnps", name="attnps")
                for h in range(H):
                    nc.tensor.matmul(
                        aps[:, h, :D], weiT[(rp, h)],
                        v_sb[rp][:, D * h : D * h + D],
                        start=(h == 0), stop=False)
                for h in range(H):
                    nc.tensor.matmul(
                        aps[:, h, D : D + 1], weiT[(rp, h)], ones_sb,
                        start=False, stop=(h == H - 1))
                rcp = smv.tile([128, H], F32, tag="rcp", name="rcp")
                nc.vector.reciprocal(rcp, aps[:, :, D])
                at = ats.tile([RP, C], F16, tag="attn", name="attn")
                nc.vector.tensor_tensor(
                    out=at.rearrange("p (h d) -> p h d", h=H),
                    in0=aps[:, :, :D],
                    in1=rcp[:, :, None].to_broadcast([128, H, D]),
                    op=ALU.mult)
                attn_sb.append(at)

            attnT = atT.tile([128, 3, ST], F16, tag="attnT", name="attnT")
            transpose_sb(attn_sb, attnT)

            x2_t = []
            for rp in range(4):
                tsl = slice(rp * RP, (rp + 1) * RP)
                ps = ffp.tile([RP, ST], F32, tag="ffps", name="ffps")
                for ct in range(3):
                    nc.tensor.matmul(
                        ps[:, :C], attnT[:, ct, tsl], wp_sb[:, ct, :],
                        start=(ct == 0), stop=(ct == 2))
                x2 = x2p.tile([RP, C], F16, tag="x2", name="x2")
                nc.vector.tensor_tensor(out=x2, in0=ps[:, :C], in1=xpb[rp], op=ALU.add)
                x2_t.append(x2)

            h2 = layer_norm(x2_t, "ln2g", "ln2b")
            h2T = h2p.tile([128, 3, ST], F16, tag="h2T", name="h2T")
            transpose_sb(h2, h2T)
            h2T8 = h2p.tile([128, 3, ST], FP8, tag="h2T8", name="h2T8")
            nc.gpsimd.tensor_copy(out=h2T8, in_=h2T)
            x2T = h2p.tile([128, 3, ST], F16, tag="x2T", name="x2T")
            transpose_sb(x2_t, x2T)
            S["x2T"] = x2T
            S["h2T8"] = h2T8

        def ffn1(st, S):
            h2T8 = S["h2T8"]
            rhs_a = h2T8[:, 0:2, :]
            rhs_b = h2T8[:, 2, :][:, None, :].to_broadcast([128, 2, ST])
            rl = [rlu.tile([128, ST], F16, tag=f"rl{mt}", name=f"rl{mt}")
                  for mt in range(12)]
            for mt in range(12):
                ps = ffp.tile([128, ST], F32, tag="ffps", name="ffps")
                nc.tensor.matmul(ps, w1a_sb[:, :, mt * 128 : (mt + 1) * 128],
                                 rhs_a, start=True, stop=False, perf_mode=DR)
                nc.tensor.matmul(ps, w1b_sb[:, :, mt * 128 : (mt + 1) * 128],
                                 rhs_b, start=False, stop=True, perf_mode=DR)
                nc.scalar.activation(
                    out=rl[mt], in_=ps, func=AF.Relu,
                    scale=1.0 / S1,
                    bias=(0.0 if trivial_bias else b1_sb[:, mt : mt + 1]))
            S["rl"] = rl

        def ffn2_out(st, S):
            rl, x2T = S["rl"], S["x2T"]
            for mt in range(3):
                ps = ffp.tile([128, ST], F32, tag="ffps", name="ffps")
                for kt in range(12):
                    nc.tensor.matmul(ps, w2_sb[:, kt, mt * 128 : (mt + 1) * 128],
                                     rl[kt], start=(kt == 0), stop=(kt == 11))
                # out stays XS-scaled in f16 (host divides by XS)
                ot = oub.tile([128, ST], F16, tag="out", name="out")
                nc.vector.tensor_tensor(out=ot, in0=ps, in1=x2T[:, mt, :],
                                        op=ALU.add)
                if not trivial_bias:
                    nc.gpsimd.tensor_scalar_add(ot, ot,
                                                b2c_sb[:, mt : mt + 1])
                nc.sync.dma_start(
                    out=out_d[mt * 128 : (mt + 1) * 128,
                              st * ST : (st + 1) * ST],
                    in_=ot)

        def run_body():
            states = {}

            def ln_stage(s):
                if 0 <= s < n_st:
                    states[s] = front_ln(s)

            for st in range(n_st + 3):
                if st == 0:
                    ln_stage(0)
                    ln_stage(1)
                # interleave scores TT (st-1) with qk projection (st): PE
                # stays dense while DVE releases score banks
                if st < n_st:
                    states[st]["qk_sb"] = []
                if 1 <= st <= n_st:
                    states[st - 1]["weiT"] = {}
                    states[st - 1]["banks"] = []
                banks = [(rp, half) for rp in range(4) for half in range(2)]
                for i in range(8):
                    if 1 <= st <= n_st:
                        emit_score_bank(states[st - 1], *banks[i])
                    if st < n_st:
                        emit_qk_tile(states[st], i, fine=(st == 0))
                if st < n_st:
                    front_v(st, states[st])
                # softmax for st-2 has most of an iteration of slack
                if 2 <= st <= n_st + 1:
                    emit_softmax(states[st - 2])
                ln_stage(st + 2)
                if st >= 3:
                    ffn1(st - 3, states[st - 3])
                if 2 <= st <= n_st + 1:
                    back1b(st - 2, states[st - 2])
                if st >= 3:
                    ffn2_out(st - 3, states.pop(st - 3))

        for _rep in range(n_rep):
            run_body()

    return nc


def ref_shard(x, inputs):
    """Numpy fp32 reference for one shard x [n, C] (n multiple of T)."""
    x = np.asarray(x, np.float32)
    Wq = np.asarray(inputs["Wq"], np.float32)
    Wk = np.asarray(inputs["Wk"], np.float32)
    Wv = np.asarray(inputs["Wv"], np.float32)
    Wp = np.asarray(inputs["Wp"], np.float32)
    W1 = np.asarray(inputs["W1"], np.float32)
    W2 = np.asarray(inputs["W2"], np.float32)
    bp = np.asarray(inputs["bp"], np.float32)
    b1 = np.asarray(inputs["b1"], np.float32)
    b2 = np.asarray(inputs["b2"], np.float32)

    def ln(v, g, b):
        mu = v.mean(-1, keepdims=True)
        var = ((v - mu) ** 2).mean(-1, keepdims=True)
        return (v - mu) / np.sqrt(var + EPS) * g + b

    B = x.shape[0] // T
    xb = x.reshape(B, T, C)
    h = ln(xb, inputs["ln1_g"], inputs["ln1_b"])
    q = np.einsum("btc,hcd->bhtd", h, Wq)
    k = np.einsum("btc,hcd->bhtd", h, Wk)
    v = np.einsum("btc,hcd->bhtd", h, Wv)
    wei = np.einsum("bhtd,bhsd->bhts", q, k) * SCALE
    causal = np.tril(np.ones((T, T), bool))
    wei = np.where(causal, wei, -np.inf)
    wei = wei - wei.max(-1, keepdims=True)
    wei = np.exp(wei)
    wei = wei / wei.sum(-1, keepdims=True)
    attn = np.einsum("bhts,bhsd->bhtd", wei, v)
    attn = attn.transpose(0, 2, 1, 3).reshape(B, T, C)
    xb = attn @ Wp + bp + xb
    h2 = ln(xb, inputs["ln2_g"], inputs["ln2_b"])
    ff = np.maximum(h2 @ W1 + b1, 0.0) @ W2 + b2
    return (ff + xb).reshape(-1, C)


# ---------------------------------------------------------------------------
# BIR post-processing: split excess sync waits onto same-engine NoOps.
WAIT_LIMITS = {"NoOp": 1, "Drain": 1, "EventSemaphore": 1, "Branch": 1,
               "DmaTransposeAnt": 1}
WAIT_LIMIT_DEFAULT = 1
_wsplit_n = [0]


def fix_bir_json(raw: bytes) -> bytes:
    import orjson
    d = orjson.loads(raw)
    for fn in d["functions"]:
        for bb in fn["blocks"]:
            insts = bb["instructions"]
            out = []
            for inst in insts:
                si = inst.get("sync_info")
                ow = (si or {}).get("on_wait") or []
                lim = WAIT_LIMITS.get(inst.get("opcode"), WAIT_LIMIT_DEFAULT)
                if len(ow) > lim:
                    keep = ow[-lim:] if lim > 0 else []
                    extra = ow[: len(ow) - lim]
                    for w in extra:
                        _wsplit_n[0] += 1
                        out.append({
                            "debug": inst.get("debug", 0),
                            "engine": inst["engine"],
                            "ins": [], "outs": [],
                            "name": f"WSPLIT-{_wsplit_n[0]}",
                            "opcode": "NoOp",
                            "sync_info": {"on_update": [], "on_wait": [w]},
                        })
                    si["on_wait"] = keep
                out.append(inst)
            bb["instructions"] = out
    return orjson.dumps(d)


def wrap_to_json(nc):
    orig = nc.to_json_bytes
    nc.to_json_bytes = lambda: fix_bir_json(orig())
    return nc


# ---------------------------------------------------------------------------
# kernel entry point
N_CORES = 8
_WKEYS = ("wqk", "wv", "wp", "w1a", "w1b", "w2", "maddn4", "bp", "b1", "b2")
_CACHE = {}


def _get_nc(n_tok, triv_ln, triv_b):
    key = (n_tok, triv_ln, triv_b)
    if key not in _CACHE:
        _CACHE[key] = wrap_to_json(
            build_nc(n_tok, trivial_ln=triv_ln, trivial_bias=triv_b))
    return _CACHE[key]


def kernel(**inputs):
    from concourse.bass_utils import run_bass_kernel_spmd

    inputs = {k: np.asarray(v) for k, v in inputs.items()}
    x = np.asarray(inputs["x"], np.float32)
    B, T_, C_ = x.shape
    assert C_ == C and B % N_CORES == 0 and (B // N_CORES) * T_ % ST == 0
    n_tok = (B // N_CORES) * T_
    w = build_weights(inputs)
    triv_ln = all(np.allclose(np.asarray(inputs[k], np.float32), v)
                  for k, v in (("ln1_g", 1.0), ("ln1_b", 0.0),
                               ("ln2_g", 1.0), ("ln2_b", 0.0)))
    triv_b = all(np.allclose(np.asarray(inputs[k], np.float32), 0.0)
                 for k in ("bp", "b1", "b2"))
    nc = _get_nc(n_tok, triv_ln, triv_b)
    xs = (x.reshape(N_CORES, n_tok, C) * XS).astype(np.float16)
    base = {k: w[k] for k in _WKEYS}
    if not triv_ln:
        base.update(ln1g=w["ln1g"], ln1b=w["ln1b"],
                    ln2g=w["ln2g"], ln2b=w["ln2b"])
    in_maps = [dict(base, x=xs[i]) for i in range(N_CORES)]
    res = run_bass_kernel_spmd(nc, in_maps, core_ids=list(range(N_CORES)))
    # out is channel-major [C, n_tok] f16 per core
    out = np.stack([res.results[i]["out"] for i in range(N_CORES)])
    out = out.transpose(0, 2, 1).astype(np.float32) / XS  # [cores, n_tok, C]
    return out.reshape(B, T_, C_)
